# revision 35
# baseline (speedup 1.0000x reference)
"""HDMNet (BiMap -> LogEig -> Linear) Trainium2 kernel, 8-core data-parallel.

Math: y_b = W^T x_b W (30x30 SPD), logm(y_b) approximated by a degree-12
polynomial in the Chebyshev variable s = alpha*y + beta*I, evaluated with a
Paterson-Stockmeyer block scheme:
    p(s) = q0(s) + T4(s)*q1(s) + T4(s)^2*q2'(s)
with q0,q1 cubic and q2' quartic Chebyshev combinations (coefficients
LS-fit on the actual eigenvalue distribution, fp16-rounded). Only 5
per-item 30x30 matrix products (T2,T3,T4 recurrence + 2 Horner levels)
instead of 18 Clenshaw steps; the scalar-coefficient combinations are
done as whole-chunk matmuls with constant c*I stationaries.

Per-item products use a block-diagonal [128x128] stationary holding 4
items' matrices, so one LDWEIGHTS+MATMUL pair covers 4 items.

Sharding: batch 8192 -> 1024 per NeuronCore; W / lin_w replicated.
"""
import os
import numpy as np

NCORES = 8
B = 8192
DIM, K, CLS = 93, 30, 117
CHUNK = 64
SLOTS = CHUNK // 4          # 16 slots of 4 stacked items
FREEW = SLOTS * K           # 480
A_LO, A_HI = 0.076, 3.51

# Device constants for the block scheme, order:
# [(2,0)..(2,4), (1,0)..(1,3), (0,0)..(0,3)]  (level i, Cheb index k)
# Level-2 (top) constants store b/2; lower levels store b (their x0.5
# evacuation compensates the doubled stationary W' = 2*T4).
CDEV = [
    -0.015716552734375,
    0.0946044921875,
    0.0018758773803710938,
    0.0304412841796875,
    0.255859375,
    -0.202880859375,
    0.131591796875,
    0.18896484375,
    1.35546875,
    -0.395751953125,
]
NQ = len(CDEV)              # 10 constant-stationary matmuls

LAST_EXEC_TIME_NS = None


def _host_consts(W, lin_w, alpha, beta):
    f16 = np.float16
    wt = (np.sqrt(2.0 * alpha) * W).astype(f16)                 # [93,30]

    # stacked identity pattern [128, FREEW]: 2*I at each (group, slot)
    idp2 = np.zeros((128, FREEW), np.float32)
    eye2 = 2.0 * np.eye(K, dtype=np.float32)
    for r in range(4):
        for s in range(SLOTS):
            idp2[32 * r:32 * r + K, K * s:K * s + K] = eye2
    bet2 = (beta * idp2).astype(np.float32)                     # 2*beta*I stacked
    idp2_16 = idp2.astype(f16)

    # wide block-diag 2*beta*I pattern [128, SLOTS*128]
    bdb2 = np.zeros((128, SLOTS * 128), f16)
    for r in range(4):
        for s in range(SLOTS):
            bdb2[32 * r:32 * r + K, s * 128 + 32 * r:s * 128 + 32 * r + K] = \
                (2.0 * beta * np.eye(K)).astype(f16)

    # constant-diagonal stationaries [128, NQ*128]
    cd = np.zeros((128, NQ * 128), f16)
    i128 = np.eye(128, dtype=np.float32)
    for j, c in enumerate(CDEV):
        cd[:, j * 128:(j + 1) * 128] = (c * i128).astype(f16)

    # q2' chain start: CDEV[0] * (2I) pattern
    qc = (CDEV[0] * idp2).astype(f16)

    # linear weights banked, CLS padded to 128: lw[32r+q, p*128+cls]
    lw = np.zeros((128, K * 128), f16)
    lwr = lin_w.reshape(CLS, K, K)          # [cls, p, q]
    blk = np.zeros((K, K * 128), np.float32)
    for p in range(K):
        blk[:, p * 128:p * 128 + CLS] = lwr[:, p, :].T          # [q, cls]
    for r in range(4):
        lw[32 * r:32 * r + K, :] = blk.astype(f16)
    return wt, idp2_16, bet2, bdb2, cd, qc, lw


def _run(x, W, lin_w, bpc):
    import concourse.bass as bass
    import concourse.bacc as bacc
    import concourse.mybir as mybir
    from concourse.tile import TileContext
    from concourse.bass_utils import run_bass_kernel_spmd

    f16, f32 = mybir.dt.float16, mybir.dt.float32
    MULT, ADD = mybir.AluOpType.mult, mybir.AluOpType.add
    nchunk = bpc // CHUNK
    alpha = 2.0 / (A_HI - A_LO)
    beta2 = -2.0 * (A_HI + A_LO) / (A_HI - A_LO)   # 2*beta

    nc = bacc.Bacc()
    xt_d = nc.dram_tensor("xt", [nchunk * DIM, CHUNK * DIM], f16,
                          kind="ExternalInput")
    wt_d = nc.dram_tensor("wt", [DIM, K], f16, kind="ExternalInput")
    idp2_d = nc.dram_tensor("idp2", [128, FREEW], f16, kind="ExternalInput")
    bet2_d = nc.dram_tensor("bet2", [128, FREEW], f32, kind="ExternalInput")
    bdb2_d = nc.dram_tensor("bdb2", [128, SLOTS * 128], f16,
                            kind="ExternalInput")
    cd_d = nc.dram_tensor("cd", [128, NQ * 128], f16, kind="ExternalInput")
    qc_d = nc.dram_tensor("qc", [128, FREEW], f16, kind="ExternalInput")
    lw_d = nc.dram_tensor("lw", [128, K * 128], f16, kind="ExternalInput")
    out_d = nc.dram_tensor("out", [CLS, bpc], f32, kind="ExternalOutput")

    with TileContext(nc) as tc:
        with tc.sbuf_pool(name="cpool", bufs=1) as cpool, \
             tc.sbuf_pool(name="xpool", bufs=8) as xpool, \
             tc.sbuf_pool(name="hpool", bufs=4) as hpool, \
             tc.sbuf_pool(name="bdpool", bufs=1) as bdpool, \
             tc.sbuf_pool(name="upool", bufs=2) as upool, \
             tc.sbuf_pool(name="spool", bufs=1) as spool:

            wt_sb = cpool.tile([DIM, K], f16, name="wt_sb")
            nc.sync.dma_start(out=wt_sb[:], in_=wt_d[:])
            idp2_sb = cpool.tile([128, FREEW], f16, name="idp2_sb")
            bet2_sb = cpool.tile([128, FREEW], f32, name="bet2_sb")
            bdb2_sb = cpool.tile([128, SLOTS * 128], f16, name="bdb2_sb")
            cd_sb = cpool.tile([128, NQ * 128], f16, name="cd_sb")
            qc_sb = cpool.tile([128, FREEW], f16, name="qc_sb")
            lw_sb = cpool.tile([128, K * 128], f16, name="lw_sb")

            def load_consts():
                # issued after the first x prefetches; split across queues
                nc.scalar.dma_start(out=bet2_sb[:], in_=bet2_d[:])
                nc.scalar.dma_start(out=idp2_sb[:], in_=idp2_d[:])
                nc.scalar.dma_start(out=bdb2_sb[:], in_=bdb2_d[:])
                nc.sync.dma_start(out=cd_sb[:], in_=cd_d[:])
                nc.sync.dma_start(out=qc_sb[:], in_=qc_d[:])
                nc.sync.dma_start(out=lw_sb[:], in_=lw_d[:])

            # double-buffered block-diag stationaries (zeros persist)
            sbd_t = [bdpool.tile([128, SLOTS * 128], f16, name=f"sbd{i}")
                     for i in range(2)]
            for t in sbd_t:
                nc.gpsimd.memset(t[:], 0.0)

            lg3 = spool.tile([128, K * bpc // 4], f16, name="lg3")
            outsb = spool.tile([CLS, bpc], f32, name="outsb")

            with tc.psum_pool(name="psA", bufs=2) as psA_pool, \
                 tc.psum_pool(name="psS", bufs=1) as psS_pool, \
                 tc.psum_pool(name="psB", bufs=2) as psB_pool, \
                 tc.psum_pool(name="psQ", bufs=1) as psQ_pool, \
                 tc.psum_pool(name="psC", bufs=1) as psC_pool:

                xins = [xpool.tile([DIM, CHUNK * DIM], f16, tag="xin",
                                   name=f"xin{cc}") for cc in range(nchunk)]

                def fetch_x(cc):
                    half = CHUNK * DIM // 2
                    r0 = cc * DIM
                    nc.sync.dma_start(
                        out=xins[cc][:, 0:half],
                        in_=xt_d[r0:r0 + DIM, 0:half])
                    nc.scalar.dma_start(
                        out=xins[cc][:, half:],
                        in_=xt_d[r0:r0 + DIM, half:])

                HW = FREEW // 2

                def halved_tt(out, a, b):
                    for h in range(2):
                        hs = slice(h * HW, (h + 1) * HW)
                        nc.vector.tensor_sub(out[:, hs], a[:, hs], b[:, hs])

                def slot_mms(ps, st, mov):
                    for s in range(SLOTS):
                        nc.tensor.matmul(
                            ps[:, s * K:(s + 1) * K],
                            st[:, s * 128:(s + 1) * 128],
                            mov[:, s * K:(s + 1) * K],
                            start=True, stop=True)

                def qconst(ps, movs, j0, kmax, close):
                    for k in range(kmax + 1):
                        nc.tensor.matmul(
                            ps[:],
                            cd_sb[:, (j0 + k) * 128:(j0 + k + 1) * 128],
                            movs[k][:],
                            start=(k == 0),
                            stop=(close and k == kmax),
                            skip_group_check=True)

                def horner_mms(ps, u3, mov):
                    for s in range(SLOTS):
                        for r in range(4):
                            p0 = 32 * r
                            nc.tensor.matmul(
                                ps[p0:p0 + K, s * K:(s + 1) * K],
                                u3[p0:p0 + K, s * K:(s + 1) * K],
                                mov[p0:p0 + K, s * K:(s + 1) * K],
                                start=False, stop=True,
                                tile_position=(p0, p0),
                                skip_group_check=True)

                def emit_A_g(c, g, psS, xin):
                    psA = psA_pool.tile([128, FREEW], f32, tag="psA",
                                        name=f"psA{c}_{g}")
                    for i in range(SLOTS):
                        bl = g * SLOTS + i
                        nc.tensor.matmul(
                            psA[0:DIM, i * K:(i + 1) * K],
                            xin[:, bl * DIM:(bl + 1) * DIM],
                            wt_sb[:],
                            start=True, stop=True)
                    hsb = hpool.tile([DIM, FREEW], f16, tag="hsb",
                                     name=f"h{c}_{g}")
                    nc.scalar.copy(out=hsb[:], in_=psA[0:DIM, :])
                    hsb3 = hsb[:].rearrange("z (i q) -> z i q", i=SLOTS)
                    for r in range(4):
                        # items bl%4==r of this group: strided 4-item moving
                        nc.tensor.matmul(
                            psS[32 * r:32 * r + K,
                                4 * g * K:4 * (g + 1) * K],
                            wt_sb[:],
                            hsb3[:, r::4, :],
                            start=True, stop=True,
                            tile_position=(0, 32 * r))

                def emit_S0(c, st):
                    # u1 = 2S stacked + block-diag scatter (DVE only)
                    psS = st['psS']
                    u1 = upool.tile([128, FREEW], f16, tag="u1",
                                    name=f"u1_{c}")
                    for h in range(2):
                        hs = slice(h * HW, (h + 1) * HW)
                        nc.vector.tensor_add(u1[:, hs], psS[:, hs],
                                             bet2_sb[:, hs])
                    sbd = sbd_t[c % 2]
                    psS3 = psS[:].rearrange("z (s q) -> z s q", s=SLOTS)
                    sbd3 = sbd[:].rearrange("z (s q) -> z s q", s=SLOTS)
                    bdb3 = bdb2_sb[:].rearrange("z (s q) -> z s q", s=SLOTS)
                    for r in range(4):
                        p0 = 32 * r
                        nc.vector.tensor_add(
                            sbd3[p0:p0 + K, :, p0:p0 + K],
                            psS3[p0:p0 + K, :, :],
                            bdb3[p0:p0 + K, :, p0:p0 + K])
                    st['u1'], st['sbd'] = u1, sbd

                def emit_S1(c, st):
                    ps2 = psB_pool.tile([128, FREEW], f32, tag="psB",
                                        name=f"ps2_{c}")
                    slot_mms(ps2, st['sbd'], st['u1'])
                    u2 = upool.tile([128, FREEW], f16, tag="u2",
                                    name=f"u2_{c}")
                    halved_tt(u2, ps2, idp2_sb)
                    st['u2'] = u2

                def emit_S2(c, st):
                    ps3 = psB_pool.tile([128, FREEW], f32, tag="psB",
                                        name=f"ps3_{c}")
                    slot_mms(ps3, st['sbd'], st['u2'])
                    u3 = upool.tile([128, FREEW], f16, tag="u3",
                                    name=f"u3_{c}")
                    halved_tt(u3, ps3, st['u1'])
                    st['u3'] = u3

                def emit_S3(c, st):
                    # A3 = q2'(S) = CDEV[0]*I + sum CDEV[k]*u_k via DVE STT
                    A3 = upool.tile([128, FREEW], f16, tag="A3",
                                    name=f"A3_{c}")
                    for h in range(2):
                        hs = slice(h * HW, (h + 1) * HW)
                        prev = qc_sb[:, hs]
                        for n, k in enumerate((1, 2, 3)):
                            if n == 2:
                                out = A3[:, hs]
                            else:
                                out = upool.tile(
                                    [128, HW], f16, tag=f"sc{h}_{n}",
                                    name=f"sc{c}_{h}_{n}")[:]
                            nc.vector.scalar_tensor_tensor(
                                out=out, in0=st['u' + str(k)][:, hs],
                                scalar=float(CDEV[k]), in1=prev,
                                op0=MULT, op1=ADD)
                            prev = out
                    st['A3'] = A3

                def emit_S4(c, st):
                    movs = [idp2_sb, st['u1'], st['u2'], st['u3']]
                    psq1 = psQ_pool.tile([128, FREEW], f32, tag="psq1",
                                         name=f"psq1_{c}")
                    qconst(psq1, movs, 4, 2, close=False)
                    horner_mms(psq1, st['u3'], st['A3'])
                    A2 = upool.tile([128, FREEW], f16, tag="A2",
                                    name=f"A2_{c}")
                    for h in range(2):
                        hs = slice(h * HW, (h + 1) * HW)
                        nc.scalar.mul(out=A2[:, hs], in_=psq1[:, hs], mul=0.5)
                    st['A2'] = A2

                def emit_S5(c, st):
                    movs = [idp2_sb, st['u1'], st['u2'], st['u3']]
                    psq0 = psQ_pool.tile([128, FREEW], f32, tag="psq0",
                                         name=f"psq0_{c}")
                    qconst(psq0, movs, 7, 2, close=False)
                    horner_mms(psq0, st['u3'], st['A2'])
                    psq03 = psq0[:].rearrange("z (s p) -> z s p", s=SLOTS)
                    lg3v = lg3[:].rearrange(
                        "z (p cc s) -> z cc s p", p=K, cc=nchunk)
                    for h in range(2):
                        nc.scalar.mul(
                            out=lg3v[:, c, h * 8:(h + 1) * 8, :],
                            in_=psq03[:, h * 8:(h + 1) * 8, :], mul=0.5)

                ncol = bpc // 4
                qcol = ncol // 4

                def emit_C(qt):
                    # linear layer for batch quarter qt (chunks 4qt..4qt+3)
                    for r in range(4):
                        psC = psC_pool.tile([128, qcol], f32, tag="psC",
                                            name=f"psC{r}_{qt}")
                        for p in range(K):
                            nc.tensor.matmul(
                                psC[:, :],
                                lw_sb[32 * r:32 * r + K,
                                      p * 128:(p + 1) * 128],
                                lg3[32 * r:32 * r + K,
                                    p * ncol + qt * qcol:
                                    p * ncol + (qt + 1) * qcol],
                                start=(p == 0), stop=(p == K - 1),
                                tile_position=(32 * r, 0))
                        nc.scalar.copy(
                            out=outsb[:, 256 * qt + r:256 * (qt + 1):4],
                            in_=psC[0:CLS, :])
                    nc.sync.dma_start(
                        out=out_d[:, 256 * qt:256 * (qt + 1)],
                        in_=outsb[:, 256 * qt:256 * (qt + 1)])

                # ---- software pipeline: A(c) interleaved with B(c-1) ----
                for cc in range(6):
                    fetch_x(cc)
                load_consts()
                states = {}
                for t in range(nchunk + 1):
                    a, b = t, t - 1
                    if a < nchunk:
                        if a + 6 < nchunk:
                            fetch_x(a + 6)
                        st_a = states[a] = {}
                        st_a['psS'] = psS_pool.tile(
                            [128, FREEW], f32, tag="psS", name=f"psS{a}")
                        emit_A_g(a, 0, st_a['psS'], xins[a])
                        if b >= 0:
                            emit_S1(b, states[b])
                        emit_A_g(a, 1, st_a['psS'], xins[a])
                        if b >= 0:
                            emit_S2(b, states[b])
                        emit_A_g(a, 2, st_a['psS'], xins[a])
                        if b >= 0:
                            emit_S3(b, states[b])
                            emit_S4(b, states[b])
                        emit_A_g(a, 3, st_a['psS'], xins[a])
                        if b >= 0:
                            emit_S5(b, states[b])
                            del states[b]['psS']
                        emit_S0(a, st_a)
                        if a in (9, 11, 13):
                            emit_C((a - 9) // 2)
                    else:
                        emit_S1(b, states[b])
                        emit_S2(b, states[b])
                        emit_S3(b, states[b])
                        emit_S4(b, states[b])
                        emit_S5(b, states[b])
                        emit_C(3)

    nc.finalize()

    # ------------- host-side input prep
    wt_np, idp2_np, bet2_np, bdb2_np, cd_np, qc_np, lw_np = _host_consts(
        W, lin_w, alpha, beta2 / 2.0)

    in_maps = []
    for ci in range(NCORES):
        xc = x[ci * bpc:(ci + 1) * bpc].astype(np.float16)  # [bpc, 93, 93]
        xtc = np.ascontiguousarray(
            xc.reshape(nchunk, CHUNK, DIM, DIM).transpose(0, 2, 1, 3)
        ).reshape(nchunk * DIM, CHUNK * DIM)
        in_maps.append({"xt": xtc, "wt": wt_np, "idp2": idp2_np,
                        "bet2": bet2_np, "bdb2": bdb2_np, "cd": cd_np,
                        "qc": qc_np, "lw": lw_np})

    res = run_bass_kernel_spmd(
        nc, in_maps, list(range(NCORES)),
        trace=bool(os.environ.get("BASS_TRACE")),
    )
    global LAST_EXEC_TIME_NS
    LAST_EXEC_TIME_NS = res.exec_time_ns
    outs = [res.results[i]["out"] for i in range(NCORES)]  # [117, bpc] each
    return np.concatenate([o.T for o in outs], axis=0)     # [B, 117]


def kernel(x, W, lin_w, lin_b):
    x = np.asarray(x, dtype=np.float32).reshape(B, DIM, DIM)
    W = np.asarray(W, dtype=np.float32)
    lin_w = np.asarray(lin_w, dtype=np.float32)
    lin_b = np.asarray(lin_b, dtype=np.float32)

    out = _run(x, W, lin_w, B // NCORES)
    return (out + lin_b[None, :]).astype(np.float32)


# revision 36
# speedup vs baseline: 1.1109x; 1.1109x over previous
"""HDMNet (BiMap -> LogEig -> Linear) Trainium2 kernel, 8-core data-parallel.

Math: y_b = W^T x_b W (30x30 SPD), logm(y_b) approximated by a degree-12
polynomial in the Chebyshev variable s = alpha*y + beta*I, evaluated with a
Paterson-Stockmeyer block scheme:
    p(s) = q0(s) + T4(s)*q1(s) + T4(s)^2*q2'(s)
with q0,q1 cubic and q2' quartic Chebyshev combinations (coefficients
LS-fit on the actual eigenvalue distribution, fp16-rounded). Only 5
per-item 30x30 matrix products (T2,T3,T4 recurrence + 2 Horner levels)
instead of 18 Clenshaw steps; the scalar-coefficient combinations are
done as whole-chunk matmuls with constant c*I stationaries.

Per-item products use a block-diagonal [128x128] stationary holding 4
items' matrices, so one LDWEIGHTS+MATMUL pair covers 4 items.

Sharding: batch 8192 -> 1024 per NeuronCore; W / lin_w replicated.
"""
import os
import numpy as np

NCORES = 8
B = 8192
DIM, K, CLS = 93, 30, 117
CHUNK = 64
SLOTS = CHUNK // 4          # 16 slots of 4 stacked items
FREEW = SLOTS * K           # 480
A_LO, A_HI = 0.076, 3.51

# Device constants for the block scheme, order:
# [(2,0)..(2,4), (1,0)..(1,3), (0,0)..(0,3)]  (level i, Cheb index k)
# Level-2 (top) constants store b/2; lower levels store b (their x0.5
# evacuation compensates the doubled stationary W' = 2*T4).
CDEV = [
    -0.015716552734375,
    0.0946044921875,
    0.0018758773803710938,
    0.0304412841796875,
    0.255859375,
    -0.202880859375,
    0.131591796875,
    0.18896484375,
    1.35546875,
    -0.395751953125,
]
NQ = len(CDEV)              # 10 constant-stationary matmuls

LAST_EXEC_TIME_NS = None


def _host_consts(W, lin_w, alpha, beta):
    f16 = np.float16
    wt = (np.sqrt(2.0 * alpha) * W).astype(f16)                 # [93,30]

    # stacked identity pattern [128, FREEW]: 2*I at each (group, slot)
    idp2 = np.zeros((128, FREEW), np.float32)
    eye2 = 2.0 * np.eye(K, dtype=np.float32)
    for r in range(4):
        for s in range(SLOTS):
            idp2[32 * r:32 * r + K, K * s:K * s + K] = eye2
    bet2 = (beta * idp2).astype(np.float32)                     # 2*beta*I stacked
    idp2_16 = idp2.astype(f16)

    # wide block-diag 2*beta*I pattern [128, SLOTS*128]
    bdb2 = np.zeros((128, SLOTS * 128), f16)
    for r in range(4):
        for s in range(SLOTS):
            bdb2[32 * r:32 * r + K, s * 128 + 32 * r:s * 128 + 32 * r + K] = \
                (2.0 * beta * np.eye(K)).astype(f16)

    # constant-diagonal stationaries [128, NQ*128]
    cd = np.zeros((128, NQ * 128), f16)
    i128 = np.eye(128, dtype=np.float32)
    for j, c in enumerate(CDEV):
        cd[:, j * 128:(j + 1) * 128] = (c * i128).astype(f16)

    # q2' chain start: CDEV[0] * (2I) pattern
    qc = (CDEV[0] * idp2).astype(f16)

    # linear weights banked, CLS padded to 128: lw[32r+q, p*128+cls]
    lw = np.zeros((128, K * 128), f16)
    lwr = lin_w.reshape(CLS, K, K)          # [cls, p, q]
    blk = np.zeros((K, K * 128), np.float32)
    for p in range(K):
        blk[:, p * 128:p * 128 + CLS] = lwr[:, p, :].T          # [q, cls]
    for r in range(4):
        lw[32 * r:32 * r + K, :] = blk.astype(f16)
    return wt, idp2_16, bet2, bdb2, cd, qc, lw


def _run(x, W, lin_w, bpc):
    import concourse.bass as bass
    import concourse.bacc as bacc
    import concourse.mybir as mybir
    from concourse.tile import TileContext
    from concourse.bass_utils import run_bass_kernel_spmd

    f16, f32 = mybir.dt.float16, mybir.dt.float32
    MULT, ADD = mybir.AluOpType.mult, mybir.AluOpType.add
    nchunk = bpc // CHUNK
    alpha = 2.0 / (A_HI - A_LO)
    beta2 = -2.0 * (A_HI + A_LO) / (A_HI - A_LO)   # 2*beta

    nc = bacc.Bacc()
    xt_d = nc.dram_tensor("xt", [nchunk * DIM, CHUNK * DIM], f16,
                          kind="ExternalInput")
    wt_d = nc.dram_tensor("wt", [DIM, K], f16, kind="ExternalInput")
    idp2_d = nc.dram_tensor("idp2", [128, FREEW], f16, kind="ExternalInput")
    bet2_d = nc.dram_tensor("bet2", [128, FREEW], f32, kind="ExternalInput")
    bdb2_d = nc.dram_tensor("bdb2", [128, SLOTS * 128], f16,
                            kind="ExternalInput")
    cd_d = nc.dram_tensor("cd", [128, NQ * 128], f16, kind="ExternalInput")
    qc_d = nc.dram_tensor("qc", [128, FREEW], f16, kind="ExternalInput")
    lw_d = nc.dram_tensor("lw", [128, K * 128], f16, kind="ExternalInput")
    out_d = nc.dram_tensor("out", [CLS, bpc], f32, kind="ExternalOutput")

    with TileContext(nc) as tc:
        with tc.sbuf_pool(name="cpool", bufs=1) as cpool, \
             tc.sbuf_pool(name="xpool", bufs=2) as xpool, \
             tc.sbuf_pool(name="hpool", bufs=4) as hpool, \
             tc.sbuf_pool(name="bdpool", bufs=1) as bdpool, \
             tc.sbuf_pool(name="upool", bufs=2) as upool, \
             tc.sbuf_pool(name="spool", bufs=1) as spool:

            wt_sb = cpool.tile([DIM, K], f16, name="wt_sb")
            nc.sync.dma_start(out=wt_sb[:], in_=wt_d[:])
            idp2_sb = cpool.tile([128, FREEW], f16, name="idp2_sb")
            bet2_sb = cpool.tile([128, FREEW], f32, name="bet2_sb")
            bdb2_sb = cpool.tile([128, SLOTS * 128], f16, name="bdb2_sb")
            cd_sb = cpool.tile([128, NQ * 128], f16, name="cd_sb")
            qc_sb = cpool.tile([128, FREEW], f16, name="qc_sb")
            lw_sb = cpool.tile([128, K * 128], f16, name="lw_sb")

            def load_consts():
                # issued after the first x prefetches; split across queues
                nc.scalar.dma_start(out=bet2_sb[:], in_=bet2_d[:])
                nc.scalar.dma_start(out=idp2_sb[:], in_=idp2_d[:])
                nc.scalar.dma_start(out=bdb2_sb[:], in_=bdb2_d[:])
                nc.sync.dma_start(out=cd_sb[:], in_=cd_d[:])
                nc.sync.dma_start(out=qc_sb[:], in_=qc_d[:])
                nc.sync.dma_start(out=lw_sb[:], in_=lw_d[:])

            # double-buffered block-diag stationaries (zeros persist)
            sbd_t = [bdpool.tile([128, SLOTS * 128], f16, name=f"sbd{i}")
                     for i in range(2)]
            for t in sbd_t:
                nc.gpsimd.memset(t[:], 0.0)

            lg3 = spool.tile([128, K * bpc // 4], f16, name="lg3")
            outsb = spool.tile([CLS, bpc], f32, name="outsb")

            with tc.psum_pool(name="psA", bufs=2) as psA_pool, \
                 tc.psum_pool(name="psS", bufs=1) as psS_pool, \
                 tc.psum_pool(name="psB", bufs=2) as psB_pool, \
                 tc.psum_pool(name="psQ", bufs=1) as psQ_pool, \
                 tc.psum_pool(name="psC", bufs=1) as psC_pool:

                xins = [xpool.tile([DIM, CHUNK * DIM], f16, tag="xin",
                                   name=f"xin{cc}") for cc in range(nchunk)]

                def fetch_x(cc):
                    half = CHUNK * DIM // 2
                    r0 = cc * DIM
                    nc.sync.dma_start(
                        out=xins[cc][:, 0:half],
                        in_=xt_d[r0:r0 + DIM, 0:half])
                    nc.scalar.dma_start(
                        out=xins[cc][:, half:],
                        in_=xt_d[r0:r0 + DIM, half:])

                HW = FREEW // 2

                def halved_tt(out, a, b):
                    for h in range(2):
                        hs = slice(h * HW, (h + 1) * HW)
                        nc.vector.tensor_sub(out[:, hs], a[:, hs], b[:, hs])

                def slot_mms(ps, st, mov):
                    for s in range(SLOTS):
                        nc.tensor.matmul(
                            ps[:, s * K:(s + 1) * K],
                            st[:, s * 128:(s + 1) * 128],
                            mov[:, s * K:(s + 1) * K],
                            start=True, stop=True)

                def qconst(ps, movs, j0, kmax, close):
                    for k in range(kmax + 1):
                        nc.tensor.matmul(
                            ps[:],
                            cd_sb[:, (j0 + k) * 128:(j0 + k + 1) * 128],
                            movs[k][:],
                            start=(k == 0),
                            stop=(close and k == kmax),
                            skip_group_check=True)

                def horner_mms(ps, u3, mov):
                    for s in range(SLOTS):
                        for r in range(4):
                            p0 = 32 * r
                            nc.tensor.matmul(
                                ps[p0:p0 + K, s * K:(s + 1) * K],
                                u3[p0:p0 + K, s * K:(s + 1) * K],
                                mov[p0:p0 + K, s * K:(s + 1) * K],
                                start=False, stop=True,
                                tile_position=(p0, p0),
                                skip_group_check=True)

                def emit_A_g(c, g, psS, xin):
                    psA = psA_pool.tile([128, FREEW], f32, tag="psA",
                                        name=f"psA{c}_{g}")
                    for i in range(SLOTS):
                        bl = g * SLOTS + i
                        nc.tensor.matmul(
                            psA[0:DIM, i * K:(i + 1) * K],
                            xin[:, bl * DIM:(bl + 1) * DIM],
                            wt_sb[:],
                            start=True, stop=True)
                    hsb = hpool.tile([DIM, FREEW], f16, tag="hsb",
                                     name=f"h{c}_{g}")
                    nc.scalar.copy(out=hsb[:], in_=psA[0:DIM, :])
                    hsb3 = hsb[:].rearrange("z (i q) -> z i q", i=SLOTS)
                    for r in range(4):
                        # items bl%4==r of this group: strided 4-item moving
                        nc.tensor.matmul(
                            psS[32 * r:32 * r + K,
                                4 * g * K:4 * (g + 1) * K],
                            wt_sb[:],
                            hsb3[:, r::4, :],
                            start=True, stop=True,
                            tile_position=(0, 32 * r))

                def emit_S0(c, st):
                    # u1 = 2S stacked + block-diag scatter (DVE only)
                    psS = st['psS']
                    u1 = upool.tile([128, FREEW], f16, tag="u1",
                                    name=f"u1_{c}")
                    for h in range(2):
                        hs = slice(h * HW, (h + 1) * HW)
                        nc.vector.tensor_add(u1[:, hs], psS[:, hs],
                                             bet2_sb[:, hs])
                    sbd = sbd_t[c % 2]
                    psS3 = psS[:].rearrange("z (s q) -> z s q", s=SLOTS)
                    sbd3 = sbd[:].rearrange("z (s q) -> z s q", s=SLOTS)
                    bdb3 = bdb2_sb[:].rearrange("z (s q) -> z s q", s=SLOTS)
                    for r in range(4):
                        p0 = 32 * r
                        nc.vector.tensor_add(
                            sbd3[p0:p0 + K, :, p0:p0 + K],
                            psS3[p0:p0 + K, :, :],
                            bdb3[p0:p0 + K, :, p0:p0 + K])
                    st['u1'], st['sbd'] = u1, sbd

                def emit_S1(c, st):
                    ps2 = psB_pool.tile([128, FREEW], f32, tag="psB",
                                        name=f"ps2_{c}")
                    slot_mms(ps2, st['sbd'], st['u1'])
                    u2 = upool.tile([128, FREEW], f16, tag="u2",
                                    name=f"u2_{c}")
                    halved_tt(u2, ps2, idp2_sb)
                    st['u2'] = u2

                def emit_S2(c, st):
                    ps3 = psB_pool.tile([128, FREEW], f32, tag="psB",
                                        name=f"ps3_{c}")
                    slot_mms(ps3, st['sbd'], st['u2'])
                    u3 = upool.tile([128, FREEW], f16, tag="u3",
                                    name=f"u3_{c}")
                    halved_tt(u3, ps3, st['u1'])
                    st['u3'] = u3

                def emit_S3(c, st):
                    # A3 = q2'(S) = CDEV[0]*I + sum CDEV[k]*u_k via DVE STT
                    A3 = upool.tile([128, FREEW], f16, tag="A3",
                                    name=f"A3_{c}")
                    for h in range(2):
                        hs = slice(h * HW, (h + 1) * HW)
                        prev = qc_sb[:, hs]
                        for n, k in enumerate((1, 2, 3)):
                            if n == 2:
                                out = A3[:, hs]
                            else:
                                out = upool.tile(
                                    [128, HW], f16, tag=f"sc{h}_{n}",
                                    name=f"sc{c}_{h}_{n}")[:]
                            nc.vector.scalar_tensor_tensor(
                                out=out, in0=st['u' + str(k)][:, hs],
                                scalar=float(CDEV[k]), in1=prev,
                                op0=MULT, op1=ADD)
                            prev = out
                    st['A3'] = A3

                def emit_S4(c, st):
                    movs = [idp2_sb, st['u1'], st['u2'], st['u3']]
                    psq1 = psQ_pool.tile([128, FREEW], f32, tag="psq1",
                                         name=f"psq1_{c}")
                    qconst(psq1, movs, 4, 2, close=False)
                    horner_mms(psq1, st['u3'], st['A3'])
                    A2 = upool.tile([128, FREEW], f16, tag="A2",
                                    name=f"A2_{c}")
                    for h in range(2):
                        hs = slice(h * HW, (h + 1) * HW)
                        nc.scalar.mul(out=A2[:, hs], in_=psq1[:, hs], mul=0.5)
                    st['A2'] = A2

                def emit_S5(c, st):
                    movs = [idp2_sb, st['u1'], st['u2'], st['u3']]
                    psq0 = psQ_pool.tile([128, FREEW], f32, tag="psq0",
                                         name=f"psq0_{c}")
                    qconst(psq0, movs, 7, 2, close=False)
                    horner_mms(psq0, st['u3'], st['A2'])
                    psq03 = psq0[:].rearrange("z (s p) -> z s p", s=SLOTS)
                    lg3v = lg3[:].rearrange(
                        "z (p cc s) -> z cc s p", p=K, cc=nchunk)
                    for h in range(2):
                        nc.scalar.mul(
                            out=lg3v[:, c, h * 8:(h + 1) * 8, :],
                            in_=psq03[:, h * 8:(h + 1) * 8, :], mul=0.5)

                ncol = bpc // 4
                qcol = ncol // 4

                def emit_C(qt):
                    # linear layer for batch quarter qt (chunks 4qt..4qt+3)
                    for r in range(4):
                        psC = psC_pool.tile([128, qcol], f32, tag="psC",
                                            name=f"psC{r}_{qt}")
                        for p in range(K):
                            nc.tensor.matmul(
                                psC[:, :],
                                lw_sb[32 * r:32 * r + K,
                                      p * 128:(p + 1) * 128],
                                lg3[32 * r:32 * r + K,
                                    p * ncol + qt * qcol:
                                    p * ncol + (qt + 1) * qcol],
                                start=(p == 0), stop=(p == K - 1),
                                tile_position=(32 * r, 0))
                        nc.scalar.copy(
                            out=outsb[:, 256 * qt + r:256 * (qt + 1):4],
                            in_=psC[0:CLS, :])
                    nc.sync.dma_start(
                        out=out_d[:, 256 * qt:256 * (qt + 1)],
                        in_=outsb[:, 256 * qt:256 * (qt + 1)])

                # ---- software pipeline: A(c) interleaved with B(c-1) ----
                for cc in range(6):
                    fetch_x(cc)
                load_consts()
                states = {}
                for t in range(nchunk + 1):
                    a, b = t, t - 1
                    if a < nchunk:
                        if a + 6 < nchunk:
                            fetch_x(a + 6)
                        st_a = states[a] = {}
                        st_a['psS'] = psS_pool.tile(
                            [128, FREEW], f32, tag="psS", name=f"psS{a}")
                        emit_A_g(a, 0, st_a['psS'], xins[a])
                        if b >= 0:
                            emit_S1(b, states[b])
                        emit_A_g(a, 1, st_a['psS'], xins[a])
                        if b >= 0:
                            emit_S2(b, states[b])
                        emit_A_g(a, 2, st_a['psS'], xins[a])
                        if b >= 0:
                            emit_S3(b, states[b])
                            emit_S4(b, states[b])
                        emit_A_g(a, 3, st_a['psS'], xins[a])
                        if b >= 0:
                            emit_S5(b, states[b])
                            del states[b]['psS']
                        emit_S0(a, st_a)
                        if a in (9, 11, 13):
                            emit_C((a - 9) // 2)
                    else:
                        emit_S1(b, states[b])
                        emit_S2(b, states[b])
                        emit_S3(b, states[b])
                        emit_S4(b, states[b])
                        emit_S5(b, states[b])
                        emit_C(3)

    nc.finalize()

    # ------------- host-side input prep
    wt_np, idp2_np, bet2_np, bdb2_np, cd_np, qc_np, lw_np = _host_consts(
        W, lin_w, alpha, beta2 / 2.0)

    in_maps = []
    for ci in range(NCORES):
        xc = x[ci * bpc:(ci + 1) * bpc].astype(np.float16)  # [bpc, 93, 93]
        xtc = np.ascontiguousarray(
            xc.reshape(nchunk, CHUNK, DIM, DIM).transpose(0, 2, 1, 3)
        ).reshape(nchunk * DIM, CHUNK * DIM)
        in_maps.append({"xt": xtc, "wt": wt_np, "idp2": idp2_np,
                        "bet2": bet2_np, "bdb2": bdb2_np, "cd": cd_np,
                        "qc": qc_np, "lw": lw_np})

    res = run_bass_kernel_spmd(
        nc, in_maps, list(range(NCORES)),
        trace=bool(os.environ.get("BASS_TRACE")),
    )
    global LAST_EXEC_TIME_NS
    LAST_EXEC_TIME_NS = res.exec_time_ns
    outs = [res.results[i]["out"] for i in range(NCORES)]  # [117, bpc] each
    return np.concatenate([o.T for o in outs], axis=0)     # [B, 117]


def kernel(x, W, lin_w, lin_b):
    x = np.asarray(x, dtype=np.float32).reshape(B, DIM, DIM)
    W = np.asarray(W, dtype=np.float32)
    lin_w = np.asarray(lin_w, dtype=np.float32)
    lin_b = np.asarray(lin_b, dtype=np.float32)

    out = _run(x, W, lin_w, B // NCORES)
    return (out + lin_b[None, :]).astype(np.float32)


# revision 37
# speedup vs baseline: 1.1272x; 1.0147x over previous
"""HDMNet (BiMap -> LogEig -> Linear) Trainium2 kernel, 8-core data-parallel.

Math: y_b = W^T x_b W (30x30 SPD), logm(y_b) approximated by a degree-12
polynomial in the Chebyshev variable s = alpha*y + beta*I, evaluated with a
Paterson-Stockmeyer block scheme:
    p(s) = q0(s) + T4(s)*q1(s) + T4(s)^2*q2'(s)
with q0,q1 cubic and q2' quartic Chebyshev combinations (coefficients
LS-fit on the actual eigenvalue distribution, fp16-rounded). Only 5
per-item 30x30 matrix products (T2,T3,T4 recurrence + 2 Horner levels)
instead of 18 Clenshaw steps; the scalar-coefficient combinations are
done as whole-chunk matmuls with constant c*I stationaries.

Per-item products use a block-diagonal [128x128] stationary holding 4
items' matrices, so one LDWEIGHTS+MATMUL pair covers 4 items.

Sharding: batch 8192 -> 1024 per NeuronCore; W / lin_w replicated.
"""
import os
import numpy as np

NCORES = 8
B = 8192
DIM, K, CLS = 93, 30, 117
CHUNK = 64
SLOTS = CHUNK // 4          # 16 slots of 4 stacked items
FREEW = SLOTS * K           # 480
A_LO, A_HI = 0.076, 3.51

# Device constants for the block scheme, order:
# [(2,0)..(2,4), (1,0)..(1,3), (0,0)..(0,3)]  (level i, Cheb index k)
# Level-2 (top) constants store b/2; lower levels store b (their x0.5
# evacuation compensates the doubled stationary W' = 2*T4).
CDEV = [
    -0.015716552734375,
    0.0946044921875,
    0.0018758773803710938,
    0.0304412841796875,
    0.255859375,
    -0.202880859375,
    0.131591796875,
    0.18896484375,
    1.35546875,
    -0.395751953125,
]
NQ = len(CDEV)              # 10 constant-stationary matmuls

LAST_EXEC_TIME_NS = None


def _host_consts(W, lin_w, alpha, beta):
    f16 = np.float16
    wt = (np.sqrt(2.0 * alpha) * W).astype(f16)                 # [93,30]

    # stacked identity pattern [128, FREEW]: 2*I at each (group, slot)
    idp2 = np.zeros((128, FREEW), np.float32)
    eye2 = 2.0 * np.eye(K, dtype=np.float32)
    for r in range(4):
        for s in range(SLOTS):
            idp2[32 * r:32 * r + K, K * s:K * s + K] = eye2
    bet2 = (beta * idp2).astype(np.float32)                     # 2*beta*I stacked
    idp2_16 = idp2.astype(f16)

    # wide block-diag 2*beta*I pattern [128, SLOTS*128]
    bdb2 = np.zeros((128, SLOTS * 128), f16)
    for r in range(4):
        for s in range(SLOTS):
            bdb2[32 * r:32 * r + K, s * 128 + 32 * r:s * 128 + 32 * r + K] = \
                (2.0 * beta * np.eye(K)).astype(f16)

    # constant-diagonal stationaries [128, NQ*128]
    cd = np.zeros((128, NQ * 128), f16)
    i128 = np.eye(128, dtype=np.float32)
    for j, c in enumerate(CDEV):
        cd[:, j * 128:(j + 1) * 128] = (c * i128).astype(f16)

    # q2' chain start: CDEV[0] * (2I) pattern
    qc = (CDEV[0] * idp2).astype(f16)

    # linear weights banked, CLS padded to 128: lw[32r+q, p*128+cls]
    lw = np.zeros((128, K * 128), f16)
    lwr = lin_w.reshape(CLS, K, K)          # [cls, p, q]
    blk = np.zeros((K, K * 128), np.float32)
    for p in range(K):
        blk[:, p * 128:p * 128 + CLS] = lwr[:, p, :].T          # [q, cls]
    for r in range(4):
        lw[32 * r:32 * r + K, :] = blk.astype(f16)
    return wt, idp2_16, bet2, bdb2, cd, qc, lw


def _run(x, W, lin_w, bpc):
    import concourse.bass as bass
    import concourse.bacc as bacc
    import concourse.mybir as mybir
    from concourse.tile import TileContext
    from concourse.bass_utils import run_bass_kernel_spmd

    f16, f32 = mybir.dt.float16, mybir.dt.float32
    MULT, ADD = mybir.AluOpType.mult, mybir.AluOpType.add
    nchunk = bpc // CHUNK
    alpha = 2.0 / (A_HI - A_LO)
    beta2 = -2.0 * (A_HI + A_LO) / (A_HI - A_LO)   # 2*beta

    nc = bacc.Bacc()
    xt_d = nc.dram_tensor("xt", [nchunk * DIM, CHUNK * DIM], f16,
                          kind="ExternalInput")
    wt_d = nc.dram_tensor("wt", [DIM, K], f16, kind="ExternalInput")
    idp2_d = nc.dram_tensor("idp2", [128, FREEW], f16, kind="ExternalInput")
    bet2_d = nc.dram_tensor("bet2", [128, FREEW], f32, kind="ExternalInput")
    bdb2_d = nc.dram_tensor("bdb2", [128, SLOTS * 128], f16,
                            kind="ExternalInput")
    cd_d = nc.dram_tensor("cd", [128, NQ * 128], f16, kind="ExternalInput")
    qc_d = nc.dram_tensor("qc", [128, FREEW], f16, kind="ExternalInput")
    lw_d = nc.dram_tensor("lw", [128, K * 128], f16, kind="ExternalInput")
    out_d = nc.dram_tensor("out", [CLS, bpc], f32, kind="ExternalOutput")

    with TileContext(nc) as tc:
        with tc.sbuf_pool(name="cpool", bufs=1) as cpool, \
             tc.sbuf_pool(name="xpool", bufs=3) as xpool, \
             tc.sbuf_pool(name="hpool", bufs=4) as hpool, \
             tc.sbuf_pool(name="bdpool", bufs=1) as bdpool, \
             tc.sbuf_pool(name="upool", bufs=2) as upool, \
             tc.sbuf_pool(name="spool", bufs=1) as spool:

            wt_sb = cpool.tile([DIM, K], f16, name="wt_sb")
            nc.sync.dma_start(out=wt_sb[:], in_=wt_d[:])
            idp2_sb = cpool.tile([128, FREEW], f16, name="idp2_sb")
            bet2_sb = cpool.tile([128, FREEW], f32, name="bet2_sb")
            bdb2_sb = cpool.tile([128, SLOTS * 128], f16, name="bdb2_sb")
            cd_sb = cpool.tile([128, NQ * 128], f16, name="cd_sb")
            qc_sb = cpool.tile([128, FREEW], f16, name="qc_sb")
            lw_sb = cpool.tile([128, K * 128], f16, name="lw_sb")

            def load_consts():
                # issued after the first x prefetches; split across queues
                nc.scalar.dma_start(out=bet2_sb[:], in_=bet2_d[:])
                nc.scalar.dma_start(out=idp2_sb[:], in_=idp2_d[:])
                nc.scalar.dma_start(out=bdb2_sb[:], in_=bdb2_d[:])
                nc.sync.dma_start(out=cd_sb[:], in_=cd_d[:])
                nc.sync.dma_start(out=qc_sb[:], in_=qc_d[:])
                nc.sync.dma_start(out=lw_sb[:], in_=lw_d[:])

            # double-buffered block-diag stationaries (zeros persist)
            sbd_t = [bdpool.tile([128, SLOTS * 128], f16, name=f"sbd{i}")
                     for i in range(2)]
            for t in sbd_t:
                nc.gpsimd.memset(t[:], 0.0)

            lg3 = spool.tile([128, K * bpc // 4], f16, name="lg3")
            outsb = spool.tile([CLS, bpc], f32, name="outsb")

            with tc.psum_pool(name="psA", bufs=2) as psA_pool, \
                 tc.psum_pool(name="psS", bufs=1) as psS_pool, \
                 tc.psum_pool(name="psB", bufs=2) as psB_pool, \
                 tc.psum_pool(name="psQ", bufs=1) as psQ_pool, \
                 tc.psum_pool(name="psC", bufs=1) as psC_pool:

                xins = [xpool.tile([DIM, CHUNK * DIM], f16, tag="xin",
                                   name=f"xin{cc}") for cc in range(nchunk)]

                def fetch_x(cc):
                    half = CHUNK * DIM // 2
                    r0 = cc * DIM
                    nc.sync.dma_start(
                        out=xins[cc][:, 0:half],
                        in_=xt_d[r0:r0 + DIM, 0:half])
                    nc.scalar.dma_start(
                        out=xins[cc][:, half:],
                        in_=xt_d[r0:r0 + DIM, half:])

                HW = FREEW // 2

                def halved_tt(out, a, b):
                    for h in range(2):
                        hs = slice(h * HW, (h + 1) * HW)
                        nc.vector.tensor_sub(out[:, hs], a[:, hs], b[:, hs])

                def slot_mms(ps, st, mov):
                    for s in range(SLOTS):
                        nc.tensor.matmul(
                            ps[:, s * K:(s + 1) * K],
                            st[:, s * 128:(s + 1) * 128],
                            mov[:, s * K:(s + 1) * K],
                            start=True, stop=True)

                def qconst(ps, movs, j0, kmax, close):
                    for k in range(kmax + 1):
                        nc.tensor.matmul(
                            ps[:],
                            cd_sb[:, (j0 + k) * 128:(j0 + k + 1) * 128],
                            movs[k][:],
                            start=(k == 0),
                            stop=(close and k == kmax),
                            skip_group_check=True)

                def horner_mms(ps, u3, mov):
                    for s in range(SLOTS):
                        for r in range(4):
                            p0 = 32 * r
                            nc.tensor.matmul(
                                ps[p0:p0 + K, s * K:(s + 1) * K],
                                u3[p0:p0 + K, s * K:(s + 1) * K],
                                mov[p0:p0 + K, s * K:(s + 1) * K],
                                start=False, stop=True,
                                tile_position=(p0, p0),
                                skip_group_check=True)

                def emit_A_g(c, g, psS, xin):
                    psA = psA_pool.tile([128, FREEW], f32, tag="psA",
                                        name=f"psA{c}_{g}")
                    for i in range(SLOTS):
                        bl = g * SLOTS + i
                        nc.tensor.matmul(
                            psA[0:DIM, i * K:(i + 1) * K],
                            xin[:, bl * DIM:(bl + 1) * DIM],
                            wt_sb[:],
                            start=True, stop=True)
                    hsb = hpool.tile([DIM, FREEW], f16, tag="hsb",
                                     name=f"h{c}_{g}")
                    nc.scalar.copy(out=hsb[:], in_=psA[0:DIM, :])
                    hsb3 = hsb[:].rearrange("z (i q) -> z i q", i=SLOTS)
                    for r in range(4):
                        # items bl%4==r of this group: strided 4-item moving
                        nc.tensor.matmul(
                            psS[32 * r:32 * r + K,
                                4 * g * K:4 * (g + 1) * K],
                            wt_sb[:],
                            hsb3[:, r::4, :],
                            start=True, stop=True,
                            tile_position=(0, 32 * r))

                def emit_S0(c, st):
                    # u1 = 2S stacked + block-diag scatter (DVE only)
                    psS = st['psS']
                    u1 = upool.tile([128, FREEW], f16, tag="u1",
                                    name=f"u1_{c}")
                    for h in range(2):
                        hs = slice(h * HW, (h + 1) * HW)
                        nc.vector.tensor_add(u1[:, hs], psS[:, hs],
                                             bet2_sb[:, hs])
                    sbd = sbd_t[c % 2]
                    psS3 = psS[:].rearrange("z (s q) -> z s q", s=SLOTS)
                    sbd3 = sbd[:].rearrange("z (s q) -> z s q", s=SLOTS)
                    bdb3 = bdb2_sb[:].rearrange("z (s q) -> z s q", s=SLOTS)
                    for r in range(4):
                        p0 = 32 * r
                        nc.vector.tensor_add(
                            sbd3[p0:p0 + K, :, p0:p0 + K],
                            psS3[p0:p0 + K, :, :],
                            bdb3[p0:p0 + K, :, p0:p0 + K])
                    st['u1'], st['sbd'] = u1, sbd

                def emit_S1(c, st):
                    ps2 = psB_pool.tile([128, FREEW], f32, tag="psB",
                                        name=f"ps2_{c}")
                    slot_mms(ps2, st['sbd'], st['u1'])
                    u2 = upool.tile([128, FREEW], f16, tag="u2",
                                    name=f"u2_{c}")
                    halved_tt(u2, ps2, idp2_sb)
                    st['u2'] = u2

                def emit_S2(c, st):
                    ps3 = psB_pool.tile([128, FREEW], f32, tag="psB",
                                        name=f"ps3_{c}")
                    slot_mms(ps3, st['sbd'], st['u2'])
                    u3 = upool.tile([128, FREEW], f16, tag="u3",
                                    name=f"u3_{c}")
                    halved_tt(u3, ps3, st['u1'])
                    st['u3'] = u3

                def emit_S3(c, st):
                    # A3 = q2'(S) = CDEV[0]*I + sum CDEV[k]*u_k via DVE STT
                    A3 = upool.tile([128, FREEW], f16, tag="A3",
                                    name=f"A3_{c}")
                    for h in range(2):
                        hs = slice(h * HW, (h + 1) * HW)
                        prev = qc_sb[:, hs]
                        for n, k in enumerate((1, 2, 3)):
                            if n == 2:
                                out = A3[:, hs]
                            else:
                                out = upool.tile(
                                    [128, HW], f16, tag=f"sc{h}_{n}",
                                    name=f"sc{c}_{h}_{n}")[:]
                            nc.vector.scalar_tensor_tensor(
                                out=out, in0=st['u' + str(k)][:, hs],
                                scalar=float(CDEV[k]), in1=prev,
                                op0=MULT, op1=ADD)
                            prev = out
                    st['A3'] = A3

                def emit_S4(c, st):
                    movs = [idp2_sb, st['u1'], st['u2'], st['u3']]
                    psq1 = psQ_pool.tile([128, FREEW], f32, tag="psq1",
                                         name=f"psq1_{c}")
                    qconst(psq1, movs, 4, 2, close=False)
                    horner_mms(psq1, st['u3'], st['A3'])
                    A2 = upool.tile([128, FREEW], f16, tag="A2",
                                    name=f"A2_{c}")
                    for h in range(2):
                        hs = slice(h * HW, (h + 1) * HW)
                        nc.scalar.mul(out=A2[:, hs], in_=psq1[:, hs], mul=0.5)
                    st['A2'] = A2

                def emit_S5(c, st):
                    movs = [idp2_sb, st['u1'], st['u2'], st['u3']]
                    psq0 = psQ_pool.tile([128, FREEW], f32, tag="psq0",
                                         name=f"psq0_{c}")
                    qconst(psq0, movs, 7, 2, close=False)
                    horner_mms(psq0, st['u3'], st['A2'])
                    psq03 = psq0[:].rearrange("z (s p) -> z s p", s=SLOTS)
                    lg3v = lg3[:].rearrange(
                        "z (p cc s) -> z cc s p", p=K, cc=nchunk)
                    for h in range(2):
                        nc.scalar.mul(
                            out=lg3v[:, c, h * 8:(h + 1) * 8, :],
                            in_=psq03[:, h * 8:(h + 1) * 8, :], mul=0.5)

                ncol = bpc // 4
                qcol = ncol // 4

                def emit_C(qt):
                    # linear layer for batch quarter qt (chunks 4qt..4qt+3)
                    for r in range(4):
                        psC = psC_pool.tile([128, qcol], f32, tag="psC",
                                            name=f"psC{r}_{qt}")
                        for p in range(K):
                            nc.tensor.matmul(
                                psC[:, :],
                                lw_sb[32 * r:32 * r + K,
                                      p * 128:(p + 1) * 128],
                                lg3[32 * r:32 * r + K,
                                    p * ncol + qt * qcol:
                                    p * ncol + (qt + 1) * qcol],
                                start=(p == 0), stop=(p == K - 1),
                                tile_position=(32 * r, 0))
                        nc.scalar.copy(
                            out=outsb[:, 256 * qt + r:256 * (qt + 1):4],
                            in_=psC[0:CLS, :])
                    nc.sync.dma_start(
                        out=out_d[:, 256 * qt:256 * (qt + 1)],
                        in_=outsb[:, 256 * qt:256 * (qt + 1)])

                # ---- software pipeline: A(c) interleaved with B(c-1) ----
                for cc in range(6):
                    fetch_x(cc)
                load_consts()
                states = {}
                for t in range(nchunk + 1):
                    a, b = t, t - 1
                    if a < nchunk:
                        if a + 6 < nchunk:
                            fetch_x(a + 6)
                        st_a = states[a] = {}
                        st_a['psS'] = psS_pool.tile(
                            [128, FREEW], f32, tag="psS", name=f"psS{a}")
                        emit_A_g(a, 0, st_a['psS'], xins[a])
                        if b >= 0:
                            emit_S1(b, states[b])
                        emit_A_g(a, 1, st_a['psS'], xins[a])
                        if b >= 0:
                            emit_S2(b, states[b])
                        emit_A_g(a, 2, st_a['psS'], xins[a])
                        if b >= 0:
                            emit_S3(b, states[b])
                            emit_S4(b, states[b])
                        emit_A_g(a, 3, st_a['psS'], xins[a])
                        if b >= 0:
                            emit_S5(b, states[b])
                            del states[b]['psS']
                        emit_S0(a, st_a)
                        if a in (9, 11, 13):
                            emit_C((a - 9) // 2)
                    else:
                        emit_S1(b, states[b])
                        emit_S2(b, states[b])
                        emit_S3(b, states[b])
                        emit_S4(b, states[b])
                        emit_S5(b, states[b])
                        emit_C(3)

    nc.finalize()

    # ------------- host-side input prep
    wt_np, idp2_np, bet2_np, bdb2_np, cd_np, qc_np, lw_np = _host_consts(
        W, lin_w, alpha, beta2 / 2.0)

    in_maps = []
    for ci in range(NCORES):
        xc = x[ci * bpc:(ci + 1) * bpc].astype(np.float16)  # [bpc, 93, 93]
        xtc = np.ascontiguousarray(
            xc.reshape(nchunk, CHUNK, DIM, DIM).transpose(0, 2, 1, 3)
        ).reshape(nchunk * DIM, CHUNK * DIM)
        in_maps.append({"xt": xtc, "wt": wt_np, "idp2": idp2_np,
                        "bet2": bet2_np, "bdb2": bdb2_np, "cd": cd_np,
                        "qc": qc_np, "lw": lw_np})

    res = run_bass_kernel_spmd(
        nc, in_maps, list(range(NCORES)),
        trace=bool(os.environ.get("BASS_TRACE")),
    )
    global LAST_EXEC_TIME_NS
    LAST_EXEC_TIME_NS = res.exec_time_ns
    outs = [res.results[i]["out"] for i in range(NCORES)]  # [117, bpc] each
    return np.concatenate([o.T for o in outs], axis=0)     # [B, 117]


def kernel(x, W, lin_w, lin_b):
    x = np.asarray(x, dtype=np.float32).reshape(B, DIM, DIM)
    W = np.asarray(W, dtype=np.float32)
    lin_w = np.asarray(lin_w, dtype=np.float32)
    lin_b = np.asarray(lin_b, dtype=np.float32)

    out = _run(x, W, lin_w, B // NCORES)
    return (out + lin_b[None, :]).astype(np.float32)
